# revision 10
# baseline (speedup 1.0000x reference)
"""CPQuadRankLayer Trainium2 kernel.

Math (per node n, batch b):
  P[b,c,r]  = sum_i x[b,n,c,i] * factors[c,n,r,i]
  p         = P / sqrt(mean_r P^2 + eps)
  merged    = p0*p1*p2*p3 * gain[n]
  out[b,o]  = sum_r merged[b,r] * factor_out[n,r,o] + mean_c x[b,n,c,o]

Distribution: nodes sharded 1024 -> 8 cores x 128 nodes (all ops are
node-independent, so no replication and no collectives).

Per-core schedule: 8 groups of 16 nodes, each group processed as 8 node
pairs stacked on SBUF partitions (64 batch rows per node) so DVE/ACT ops
run at the full 128 partitions. All DMA uses natural-contiguity layouts
(>=512B runs per descriptor); the i-on-partition matmul operands are
produced on-chip with PE transposes.
"""

import numpy as np

B = 64
N = 1024
C = 4
D = 128
R = 64
NCORES = 8
NS = N // NCORES  # nodes per core
G = 16  # nodes per group
NH = NS // 2  # node pairs per core
GH = G // 2  # node pairs per group
EPS = 1e-6

_CACHE = {}


def _build_nc(repeat=1):
    import concourse.bacc as bacc
    import concourse.tile as tile
    import concourse.mybir as mybir
    from concourse.masks import make_identity

    dt = mybir.dt.float32
    Alu = mybir.AluOpType
    Act = mybir.ActivationFunctionType

    nc = bacc.Bacc()
    x = nc.declare_dram_parameter("x", [B, NS, C, D], dt, isOutput=False)
    f = nc.declare_dram_parameter("factors", [C, NS, R, D], dt, isOutput=False)
    fo = nc.declare_dram_parameter("factor_out", [NS, R, D], dt, isOutput=False)
    gain = nc.declare_dram_parameter("gain", [NS, 1], dt, isOutput=False)
    out = nc.declare_dram_parameter("out", [B, NS, D], dt, isOutput=True)

    # Views with the node axis split into (pair nh, member g2).
    x_r = x.rearrange("b (nh g2) c i -> g2 b nh c i", g2=2)
    f_r = f.rearrange("c (nh g2) r i -> (g2 r) c nh i", g2=2)
    fo_r = fo.rearrange("(nh g2) r o -> r nh g2 o", g2=2)
    out_r = out.rearrange("b (nh g2) o -> g2 b nh o", g2=2)

    with tile.TileContext(nc) as tc:
        with (
            tc.tile_pool(name="consts", bufs=1) as consts,
            tc.tile_pool(name="xpool", bufs=2) as xpool,
            tc.tile_pool(name="fpool", bufs=2) as fpool,
            tc.tile_pool(name="fopool", bufs=2) as fopool,
            tc.tile_pool(name="opool", bufs=2) as opool,
            tc.tile_pool(name="xtpool", bufs=2) as xtpool,
            tc.tile_pool(name="ftpool", bufs=2) as ftpool,
            tc.tile_pool(name="work", bufs=3) as work,
            tc.tile_pool(name="small", bufs=4) as small,
            tc.tile_pool(name="trps", bufs=2, space="PSUM") as trps,
            tc.tile_pool(name="pps", bufs=2, space="PSUM") as pps,
            tc.tile_pool(name="mtps", bufs=2, space="PSUM") as mtps,
            tc.tile_pool(name="ops", bufs=2, space="PSUM") as ops,
        ):
            identity = consts.tile([128, 128], dt)
            make_identity(nc, identity)
            eps_t = consts.tile([128, 1], dt)
            nc.vector.memset(eps_t, EPS)

            # Broadcast gain to all partitions via a K=1 outer product:
            # gb[p, n] = gain[n] for every partition p.
            ones1 = consts.tile([1, 128], dt)
            nc.vector.memset(ones1, 1.0)
            g1 = consts.tile([1, NS], dt)
            nc.sync.dma_start(out=g1, in_=gain.rearrange("n o -> o n"))
            gbp = trps.tile([128, NS], dt, tag="tr")
            nc.tensor.matmul(gbp, lhsT=ones1, rhs=g1)
            gb = consts.tile([128, NS], dt)
            nc.any.tensor_copy(gb, gbp)

            def emit_all_groups():
              for gi in range(NS // G):
                h0 = gi * GH  # first pair index of this group
                # x, pair-stacked on partitions: p = g2*64 + b.
                # Two complementary half-partition DMAs on the two HWDGE
                # rings so all 16 SDMA engines stay busy.
                x_t = xpool.tile([128, GH, C, D], dt)
                nc.sync.dma_start(out=x_t[0:64], in_=x_r[0, :, h0 : h0 + GH])
                nc.scalar.dma_start(out=x_t[64:128], in_=x_r[1, :, h0 : h0 + GH])
                # factors, pair-stacked p = g2*64 + r (affine in DRAM).
                f_t = fpool.tile([128, C, GH, D], dt)
                for c in range(C):
                    nc.sync.dma_start(
                        out=f_t[:, c], in_=f_r[:, c, h0 : h0 + GH]
                    )
                # factor_out, r on partitions 0-63 (matmul operands must
                # share a base partition with mt, which lives at base 0).
                fo_t = fopool.tile([R, GH, 2, D], dt)
                nc.scalar.dma_start(out=fo_t, in_=fo_r[:, h0 : h0 + GH])
                o_t = opool.tile([128, GH, D], dt)

                for gh in range(GH):
                    # ---- transposes: [p, i] -> [i, p] for x and factors
                    xt_t = xtpool.tile([128, C, 128], dt)
                    ft_t = ftpool.tile([128, C, 128], dt)
                    for c in range(C):
                        tp = trps.tile([128, 128], dt, tag="tr")
                        nc.tensor.matmul(tp, lhsT=x_t[:, gh, c, :], rhs=identity)
                        nc.any.tensor_copy(xt_t[:, c, :], tp)
                        tq = trps.tile([128, 128], dt, tag="tr")
                        nc.tensor.matmul(tq, lhsT=f_t[:, c, gh, :], rhs=identity)
                        nc.any.tensor_copy(ft_t[:, c, :], tq)

                    # ---- P[b,r] per (node, child): K=i matmuls
                    pp = pps.tile([128, C, R], dt)
                    for c in range(C):
                        for g2 in range(2):
                            nc.tensor.matmul(
                                pp[64 * g2 : 64 * g2 + 64, c, :],
                                lhsT=xt_t[:, c, 64 * g2 : 64 * g2 + 64],
                                rhs=ft_t[:, c, 64 * g2 : 64 * g2 + 64],
                            )

                    # single PSUM->SBUF copy; later ops read SBUF only
                    # (walrus: at most one PSUM input per instruction)
                    ppsb = work.tile([128, C, R], dt)
                    nc.any.tensor_copy(ppsb, pp)

                    # ---- RMS over r: ssq = sum_r P^2, rms = sqrt(ssq/R+eps)
                    sqs = work.tile([128, C, R], dt)
                    nc.vector.tensor_mul(sqs, ppsb, ppsb)
                    ssq = small.tile([128, C], dt)
                    nc.vector.reduce_sum(out=ssq, in_=sqs, axis=mybir.AxisListType.X)
                    rms = small.tile([128, C], dt)
                    nc.scalar.activation(
                        out=rms, in_=ssq, func=Act.Sqrt, bias=eps_t, scale=1.0 / R
                    )
                    rstd = small.tile([128, C], dt)
                    nc.vector.reciprocal(out=rstd, in_=rms)
                    sc01 = small.tile([128, 1], dt)
                    nc.vector.tensor_mul(sc01, rstd[:, 0:1], rstd[:, 1:2])
                    sc23 = small.tile([128, 1], dt)
                    nc.vector.tensor_mul(sc23, rstd[:, 2:3], rstd[:, 3:4])
                    scl = small.tile([128, 1], dt)
                    nc.vector.tensor_mul(scl, sc01, sc23)

                    # ---- merged = P0*P1*P2*P3 * (rstd0*rstd1*rstd2*rstd3)
                    m01 = work.tile([128, R], dt)
                    nc.vector.tensor_mul(m01, ppsb[:, 0, :], ppsb[:, 1, :])
                    m23 = work.tile([128, R], dt)
                    nc.vector.tensor_mul(m23, ppsb[:, 2, :], ppsb[:, 3, :])
                    mg = work.tile([128, R], dt)
                    nc.vector.tensor_mul(mg, m01, m23)
                    nc.vector.tensor_scalar_mul(out=mg, in0=mg, scalar1=scl)

                    # ---- transpose merged: [p, r] -> [r, p]; apply gain
                    mtp = mtps.tile([64, 128], dt)
                    nc.tensor.matmul(mtp, lhsT=mg, rhs=identity)
                    mt = work.tile([64, 128], dt)
                    nc.any.tensor_copy(mt, mtp)
                    for g2 in range(2):
                        col = (h0 + gh) * 2 + g2
                        nc.vector.tensor_scalar_mul(
                            out=mt[:, 64 * g2 : 64 * g2 + 64],
                            in0=mt[:, 64 * g2 : 64 * g2 + 64],
                            scalar1=gb[0:64, col : col + 1],
                        )

                    # ---- out[b,o] = merged @ factor_out : K=r matmuls
                    op = ops.tile([128, D], dt)
                    for g2 in range(2):
                        nc.tensor.matmul(
                            op[64 * g2 : 64 * g2 + 64, :],
                            lhsT=mt[:, 64 * g2 : 64 * g2 + 64],
                            rhs=fo_t[:, gh, g2, :],
                        )

                    # ---- residual: + 0.25 * sum_c x
                    t1 = work.tile([128, D], dt)
                    nc.vector.tensor_add(t1, x_t[:, gh, 0, :], x_t[:, gh, 1, :])
                    t2 = work.tile([128, D], dt)
                    nc.vector.tensor_add(t2, x_t[:, gh, 2, :], x_t[:, gh, 3, :])
                    ts = work.tile([128, D], dt)
                    nc.vector.tensor_add(ts, t1, t2)
                    xq = work.tile([128, D], dt)
                    nc.vector.tensor_scalar_mul(out=xq, in0=ts, scalar1=0.25)
                    nc.vector.tensor_add(o_t[:, gh, :], op, xq)

                # store the group's outputs (two complementary halves)
                nc.sync.dma_start(out=out_r[0, :, h0 : h0 + GH], in_=o_t[0:64])
                nc.scalar.dma_start(out=out_r[1, :, h0 : h0 + GH], in_=o_t[64:128])

            if repeat > 1:
                with tc.For_i(0, repeat, 1):
                    emit_all_groups()
            else:
                emit_all_groups()

    nc.compile()
    return nc


def _get_nc(repeat=1):
    key = ("nc", repeat)
    if key not in _CACHE:
        _CACHE[key] = _build_nc(repeat)
    return _CACHE[key]


def kernel(x, factors, factor_out, gain):
    from concourse.bass_utils import run_bass_kernel_spmd

    nc = _get_nc()
    in_maps = []
    for k in range(NCORES):
        lo, hi = k * NS, (k + 1) * NS
        in_maps.append(
            {
                "x": np.ascontiguousarray(x[:, lo:hi]),
                "factors": np.ascontiguousarray(factors[:, lo:hi]),
                "factor_out": np.ascontiguousarray(factor_out[lo:hi]),
                "gain": np.ascontiguousarray(gain[lo:hi]),
            }
        )
    res = run_bass_kernel_spmd(nc, in_maps, core_ids=list(range(NCORES)))
    return np.concatenate([res.results[k]["out"] for k in range(NCORES)], axis=1)


# revision 21
# speedup vs baseline: 2.6096x; 2.6096x over previous
"""CPQuadRankLayer Trainium2 kernel, fully host-prepacked layouts.

Math (per node n, batch b):
  P[b,c,r]  = sum_i x[b,n,c,i] * factors[c,n,r,i]
  p         = P / sqrt(mean_r P^2 + eps)
  merged    = p0*p1*p2*p3 * gain[n]
  out[b,o]  = sum_r merged[b,r] * factor_out[n,r,o] + mean_c x[b,n,c,o]

Distribution: nodes sharded 1024 -> 8 cores x 128 nodes (node-
independent: no replication, no collectives). All tensors are repacked
on the host so every DMA runs full-width with >=2KiB contiguous runs
and the contraction dims land directly on SBUF partitions (no on-chip
transposes of x or factors; only the tiny per-pair merged transpose
remains on the PE). The second matmul produces transposed output
[o, b] so the residual is applied in the same space; the packed output
is unpacked on the host.
"""

import numpy as np

B = 64
N = 1024
C = 4
D = 128
R = 64
NCORES = 8
NS = N // NCORES  # nodes per core (128)
G = 16  # nodes per group
NH = NS // 2  # node pairs per core
GH = G // 2  # node pairs per group
NG = NS // G  # groups per core (8)
OCT = NS // 8  # octets per core (16)
QUAD = NS // 4  # quads per core (32)
EPS = 1e-6

_CACHE = {}


def _build_nc(repeat=1):
    import concourse.bacc as bacc
    import concourse.tile as tile
    import concourse.mybir as mybir
    from concourse.masks import make_identity

    dt = mybir.dt.float32
    Act = mybir.ActivationFunctionType

    nc = bacc.Bacc()
    # x pre-packed: [octet, c, i, (pair4, g2, b)]
    xp = nc.declare_dram_parameter("xp", [OCT, C, D, 512], dt, isOutput=False)
    # factors pre-packed: [c, octet, i, (node8, r)]
    f = nc.declare_dram_parameter("factors_t", [C, OCT, D, 8 * R], dt, isOutput=False)
    # factor_out pre-packed: [quad, r, (node4, o)]
    fo = nc.declare_dram_parameter("factor_out_t", [QUAD, R, 4 * D], dt, isOutput=False)
    gain = nc.declare_dram_parameter("gain", [NS, 1], dt, isOutput=False)
    # packed output: [group, o, (gh, g2, b)]; host unpacks
    out = nc.declare_dram_parameter("out_t", [NG, 128, GH * D], dt, isOutput=True)

    xp_r = xp.rearrange("u c i w -> i c u w")
    f_r = f.rearrange("c u i w -> i c u w")
    fo_r = fo.rearrange("q r w -> r q w")

    with tile.TileContext(nc) as tc:
        with (
            tc.tile_pool(name="consts", bufs=1) as consts,
            tc.tile_pool(name="xpool", bufs=2) as xpool,
            tc.tile_pool(name="fpool", bufs=2) as fpool,
            tc.tile_pool(name="fopool", bufs=2) as fopool,
            tc.tile_pool(name="opool", bufs=2) as opool,
            tc.tile_pool(name="ppool", bufs=2) as ppool,
            tc.tile_pool(name="sqpool", bufs=2) as sqpool,
            tc.tile_pool(name="rpool", bufs=2) as rpool,
            tc.tile_pool(name="work", bufs=3) as work,
            tc.tile_pool(name="small", bufs=4) as small,
            tc.tile_pool(name="trps", bufs=1, space="PSUM") as trps,
            tc.tile_pool(name="pps", bufs=3, space="PSUM") as pps,
            tc.tile_pool(name="mtps", bufs=2, space="PSUM") as mtps,
            tc.tile_pool(name="ops", bufs=2, space="PSUM") as ops,
        ):
            identity = consts.tile([128, 128], dt)
            make_identity(nc, identity)
            eps_t = consts.tile([128, 1], dt)
            nc.vector.memset(eps_t, EPS)

            # gpair[p, h] = gain[2h + (p >= 64)] via two K=1 outer products
            ones1 = consts.tile([1, 128], dt)
            nc.vector.memset(ones1, 1.0)
            g1 = consts.tile([1, NS], dt)
            nc.sync.dma_start(out=g1, in_=gain.rearrange("n o -> o n"))
            g1v = g1.rearrange("o (h g2) -> o h g2", g2=2)
            gpp = trps.tile([128, NH], dt, tag="tr")
            nc.tensor.matmul(gpp[0:64, :], lhsT=ones1[:, 0:64], rhs=g1v[:, :, 0])
            nc.tensor.matmul(gpp[64:128, :], lhsT=ones1[:, 0:64], rhs=g1v[:, :, 1])
            gpair = consts.tile([128, NH], dt)
            nc.any.tensor_copy(gpair, gpp)

            def emit_all_groups():
              for gi in range(NG):
                h0 = gi * GH
                # x transposed, i on partitions, 2KiB runs, full width
                xt_g = xpool.tile([128, C, 2, 512], dt)
                for c in range(C):
                    eng = nc.sync if c % 2 == 0 else nc.scalar
                    eng.dma_start(
                        out=xt_g[:, c], in_=xp_r[:, c, 2 * gi : 2 * gi + 2]
                    )
                # factors, same structure
                f_t = fpool.tile([128, C, 2, 8 * R], dt)
                for c in range(C):
                    eng = nc.scalar if c % 2 == 0 else nc.sync
                    eng.dma_start(
                        out=f_t[:, c], in_=f_r[:, c, 2 * gi : 2 * gi + 2]
                    )
                # factor_out, r on partitions 0-63
                fo_t = fopool.tile([R, 4, 4 * D], dt)
                nc.scalar.dma_start(out=fo_t, in_=fo_r[:, 4 * gi : 4 * gi + 4])
                o_t = opool.tile([128, GH, D], dt)
                ppall = ppool.tile([128, GH, C, R], dt)

                # ---- phase 1: P matmuls staged to SBUF, two pairs
                # per PSUM bank so each copy moves 2KiB/partition
                for ghp in range(0, GH, 2):
                    pp = pps.tile([128, 2, C, R], dt)
                    for dg in range(2):
                        gh = ghp + dg
                        u = gh // 4
                        for c in range(C):
                            for g2 in range(2):
                                j = 2 * gh + g2
                                jj = j % 8
                                nc.tensor.matmul(
                                    pp[64 * g2 : 64 * g2 + 64, dg, c, :],
                                    lhsT=xt_g[:, c, u, 64 * jj : 64 * jj + 64],
                                    rhs=f_t[:, c, u, 64 * jj : 64 * jj + 64],
                                )
                    nc.scalar.copy(out=ppall[:, ghp : ghp + 2], in_=pp)

                # ---- group-batched RMS stats
                sq = sqpool.tile([128, GH, C, R], dt)
                nc.scalar.activation(out=sq, in_=ppall, func=Act.Square)
                ssq = small.tile([128, GH * C], dt)
                nc.vector.reduce_sum(
                    out=ssq,
                    in_=sq.rearrange("p gh c r -> p (gh c) r"),
                    axis=mybir.AxisListType.X,
                )
                rms = small.tile([128, GH * C], dt)
                nc.scalar.activation(
                    out=rms, in_=ssq, func=Act.Sqrt, bias=eps_t, scale=1.0 / R
                )
                rstd = small.tile([128, GH, C], dt)
                nc.vector.reciprocal(
                    out=rstd, in_=rms.rearrange("p (gh c) -> p gh c", c=C)
                )
                sa = small.tile([128, GH], dt)
                nc.vector.tensor_mul(sa, rstd[:, :, 0], rstd[:, :, 1])
                sb = small.tile([128, GH], dt)
                nc.vector.tensor_mul(sb, rstd[:, :, 2], rstd[:, :, 3])
                sab = small.tile([128, GH], dt)
                nc.vector.tensor_mul(sab, sa, sb)
                scl2 = small.tile([128, GH], dt)
                nc.vector.tensor_mul(scl2, sab, gpair[:, h0 : h0 + GH])

                # ---- group-batched residual 0.25*sum_c x (transposed space)
                rt1 = rpool.tile([128, 2, 512], dt)
                nc.vector.tensor_add(rt1, xt_g[:, 0], xt_g[:, 1])
                rt2 = rpool.tile([128, 2, 512], dt)
                nc.vector.tensor_add(rt2, xt_g[:, 2], xt_g[:, 3])
                rts = rpool.tile([128, 2, 512], dt)
                nc.vector.tensor_add(rts, rt1, rt2)
                xq = rpool.tile([128, 2, 512], dt)
                nc.vector.tensor_scalar_mul(out=xq, in0=rts, scalar1=0.25)

                # ---- phase 2 per pair: merged, second matmul (transposed)
                for gh in range(GH):
                    u, gh4 = gh // 4, gh % 4
                    m01 = work.tile([128, R], dt)
                    nc.vector.tensor_mul(m01, ppall[:, gh, 0], ppall[:, gh, 1])
                    m23 = work.tile([128, R], dt)
                    nc.vector.tensor_mul(m23, ppall[:, gh, 2], ppall[:, gh, 3])
                    mg = work.tile([128, R], dt)
                    nc.vector.tensor_mul(mg, m01, m23)
                    nc.vector.tensor_scalar_mul(
                        out=mg, in0=mg, scalar1=scl2[:, gh : gh + 1]
                    )

                    # transpose merged [(g2 b), r] -> [r, (g2 b)]
                    mtp = mtps.tile([64, 128], dt)
                    nc.tensor.matmul(mtp, lhsT=mg, rhs=identity)
                    mt = work.tile([64, 128], dt)
                    nc.scalar.copy(out=mt, in_=mtp)

                    # out_T[o, b] per node, pair-stacked along free
                    op = ops.tile([128, D], dt)
                    for g2 in range(2):
                        j = 2 * gh + g2
                        q, j4 = j // 4, j % 4
                        nc.tensor.matmul(
                            op[:, 64 * g2 : 64 * g2 + 64],
                            lhsT=fo_t[:, q, 128 * j4 : 128 * j4 + 128],
                            rhs=mt[:, 64 * g2 : 64 * g2 + 64],
                        )

                    nc.vector.tensor_add(
                        o_t[:, gh, :], op, xq[:, u, 128 * gh4 : 128 * gh4 + 128]
                    )

                # single full-width packed store, 4KiB runs
                nc.sync.dma_start(
                    out=out[gi], in_=o_t.rearrange("p gh o -> p (gh o)")
                )

            if repeat > 1:
                with tc.For_i(0, repeat, 1):
                    emit_all_groups()
            else:
                emit_all_groups()

    nc.compile()
    return nc


def _get_nc(repeat=1):
    key = ("nc", repeat)
    if key not in _CACHE:
        _CACHE[key] = _build_nc(repeat)
    return _CACHE[key]


def _pack_x(x):
    # [B, N, C, D] -> [N//8, C, D, 512] ; n = oct*8 + jj, col = jj*64 + b
    a = x.reshape(B, N // 8, 8, C, D)
    a = np.transpose(a, (1, 3, 4, 2, 0))  # [oct, c, i, jj, b]
    return np.ascontiguousarray(a.reshape(N // 8, C, D, 512))


def _pack_factors(factors):
    # [4, N, R, D] -> [C, N//8, D, 8*R]
    f = factors.reshape(C, N // 8, 8, R, D)
    f = np.transpose(f, (0, 1, 4, 2, 3))  # [c, oct, i, node8, r]
    return np.ascontiguousarray(f.reshape(C, N // 8, D, 8 * R))


def _pack_factor_out(factor_out):
    # [N, R, D] -> [N//4, R, 4*D]
    q = factor_out.reshape(N // 4, 4, R, D)
    q = np.transpose(q, (0, 2, 1, 3))  # [quad, r, node4, o]
    return np.ascontiguousarray(q.reshape(N // 4, R, 4 * D))


def _unpack_out(res_t):
    # [NG, 128(o), GH*D] with col = gh*128 + g2*64 + b -> [B, NS, D]
    a = res_t.reshape(NG, 128, GH, 2, 64)  # [gi, o, gh, g2, b]
    a = np.transpose(a, (4, 0, 2, 3, 1))  # [b, gi, gh, g2, o]
    return np.ascontiguousarray(a.reshape(64, NS, D))


def kernel(x, factors, factor_out, gain):
    from concourse.bass_utils import run_bass_kernel_spmd

    nc = _get_nc()
    x_packed = _pack_x(np.asarray(x))
    f_packed = _pack_factors(np.asarray(factors))
    fo_packed = _pack_factor_out(np.asarray(factor_out))
    in_maps = []
    for k in range(NCORES):
        lo, hi = k * NS, (k + 1) * NS
        in_maps.append(
            {
                "xp": np.ascontiguousarray(x_packed[k * OCT : (k + 1) * OCT]),
                "factors_t": np.ascontiguousarray(f_packed[:, k * OCT : (k + 1) * OCT]),
                "factor_out_t": np.ascontiguousarray(
                    fo_packed[k * QUAD : (k + 1) * QUAD]
                ),
                "gain": np.ascontiguousarray(gain[lo:hi]),
            }
        )
    res = run_bass_kernel_spmd(nc, in_maps, core_ids=list(range(NCORES)))
    return np.concatenate(
        [_unpack_out(res.results[k]["out_t"]) for k in range(NCORES)], axis=1
    )


# revision 24
# speedup vs baseline: 2.8686x; 1.0993x over previous
"""CPQuadRankLayer Trainium2 kernel, fully host-prepacked layouts.

Math (per node n, batch b):
  P[b,c,r]  = sum_i x[b,n,c,i] * factors[c,n,r,i]
  p         = P / sqrt(mean_r P^2 + eps)
  merged    = p0*p1*p2*p3 * gain[n]
  out[b,o]  = sum_r merged[b,r] * factor_out[n,r,o] + mean_c x[b,n,c,o]

Distribution: nodes sharded 1024 -> 8 cores x 128 nodes (node-
independent: no replication, no collectives). All tensors are repacked
on the host so every DMA runs full-width with >=2KiB contiguous runs
and the contraction dims land directly on SBUF partitions (no on-chip
transposes of x or factors; only the tiny per-pair merged transpose
remains on the PE). The second matmul produces transposed output
[o, b] so the residual is applied in the same space; the packed output
is unpacked on the host.
"""

import numpy as np

B = 64
N = 1024
C = 4
D = 128
R = 64
NCORES = 8
NS = N // NCORES  # nodes per core (128)
G = 16  # nodes per group
NH = NS // 2  # node pairs per core
GH = G // 2  # node pairs per group
NG = NS // G  # groups per core (8)
OCT = NS // 8  # octets per core (16)
QUAD = NS // 4  # quads per core (32)
EPS = 1e-6

_CACHE = {}


def _build_nc(repeat=1):
    import concourse.bacc as bacc
    import concourse.tile as tile
    import concourse.mybir as mybir
    from concourse.masks import make_identity

    dt = mybir.dt.float32
    Act = mybir.ActivationFunctionType

    nc = bacc.Bacc()
    # x pre-packed: [group, c, i, (node16, b)] -> 4KiB runs
    xp = nc.declare_dram_parameter("xp", [NG, C, D, 1024], dt, isOutput=False)
    # factors pre-packed: [c, group, i, (node16, r)] -> 4KiB runs
    f = nc.declare_dram_parameter("factors_t", [C, NG, D, 1024], dt, isOutput=False)
    # factor_out pre-packed: [octet, r, (node8, o)] -> 4KiB runs
    fo = nc.declare_dram_parameter("factor_out_t", [OCT, R, 8 * D], dt, isOutput=False)
    gain = nc.declare_dram_parameter("gain", [NS, 1], dt, isOutput=False)
    # packed output: [group, o, (gh, g2, b)]; host unpacks
    out = nc.declare_dram_parameter("out_t", [NG, 128, GH * D], dt, isOutput=True)

    xp_r = xp.rearrange("g c i w -> i g c w")
    f_r = f.rearrange("c g i w -> i g c w")
    fo_r = fo.rearrange("u r w -> r u w")

    with tile.TileContext(nc) as tc:
        with (
            tc.tile_pool(name="consts", bufs=1) as consts,
            tc.tile_pool(name="xpool", bufs=3) as xpool,
            tc.tile_pool(name="fpool", bufs=3) as fpool,
            tc.tile_pool(name="fopool", bufs=2) as fopool,
            tc.tile_pool(name="opool", bufs=2) as opool,
            tc.tile_pool(name="ppool", bufs=2) as ppool,
            tc.tile_pool(name="sqpool", bufs=2) as sqpool,
            tc.tile_pool(name="rpool", bufs=2) as rpool,
            tc.tile_pool(name="work", bufs=3) as work,
            tc.tile_pool(name="small", bufs=4) as small,
            tc.tile_pool(name="trps", bufs=1, space="PSUM") as trps,
            tc.tile_pool(name="pps", bufs=3, space="PSUM") as pps,
            tc.tile_pool(name="mtps", bufs=2, space="PSUM") as mtps,
            tc.tile_pool(name="ops", bufs=2, space="PSUM") as ops,
        ):
            identity = consts.tile([128, 128], dt)
            make_identity(nc, identity)
            eps_t = consts.tile([128, 1], dt)
            nc.vector.memset(eps_t, EPS)

            # gpair[p, h] = gain[2h + (p >= 64)] via two K=1 outer products
            ones1 = consts.tile([1, 128], dt)
            nc.vector.memset(ones1, 1.0)
            g1 = consts.tile([1, NS], dt)
            nc.sync.dma_start(out=g1, in_=gain.rearrange("n o -> o n"))
            g1v = g1.rearrange("o (h g2) -> o h g2", g2=2)
            gpp = trps.tile([128, NH], dt, tag="tr")
            nc.tensor.matmul(gpp[0:64, :], lhsT=ones1[:, 0:64], rhs=g1v[:, :, 0])
            nc.tensor.matmul(gpp[64:128, :], lhsT=ones1[:, 0:64], rhs=g1v[:, :, 1])
            gpair = consts.tile([128, NH], dt)
            nc.any.tensor_copy(gpair, gpp)

            def emit_all_groups():
              for gi in range(NG):
                h0 = gi * GH
                # single 2MiB load per tensor, 4KiB runs, full width
                xt_g = xpool.tile([128, C, 1024], dt)
                nc.sync.dma_start(out=xt_g, in_=xp_r[:, gi])
                f_t = fpool.tile([128, C, 1024], dt)
                nc.scalar.dma_start(out=f_t, in_=f_r[:, gi])
                # factor_out, r on partitions 0-63
                fo_t = fopool.tile([R, 2, 8 * D], dt)
                nc.scalar.dma_start(out=fo_t, in_=fo_r[:, 2 * gi : 2 * gi + 2])
                o_t = opool.tile([128, GH, D], dt)
                ppall = ppool.tile([128, GH, C, R], dt)

                # ---- phase 1: P matmuls staged to SBUF, two pairs
                # per PSUM bank so each copy moves 2KiB/partition
                for ghp in range(0, GH, 2):
                    pp = pps.tile([128, 2, C, R], dt)
                    for dg in range(2):
                        gh = ghp + dg
                        for c in range(C):
                            for g2 in range(2):
                                j = 2 * gh + g2
                                nc.tensor.matmul(
                                    pp[64 * g2 : 64 * g2 + 64, dg, c, :],
                                    lhsT=xt_g[:, c, 64 * j : 64 * j + 64],
                                    rhs=f_t[:, c, 64 * j : 64 * j + 64],
                                )
                    nc.scalar.copy(out=ppall[:, ghp : ghp + 2], in_=pp)

                # ---- group-batched RMS stats
                sq = sqpool.tile([128, GH, C, R], dt)
                nc.scalar.activation(out=sq, in_=ppall, func=Act.Square)
                ssq = small.tile([128, GH * C], dt)
                nc.vector.reduce_sum(
                    out=ssq,
                    in_=sq.rearrange("p gh c r -> p (gh c) r"),
                    axis=mybir.AxisListType.X,
                )
                rms = small.tile([128, GH * C], dt)
                nc.scalar.activation(
                    out=rms, in_=ssq, func=Act.Sqrt, bias=eps_t, scale=1.0 / R
                )
                rstd = small.tile([128, GH, C], dt)
                nc.vector.reciprocal(
                    out=rstd, in_=rms.rearrange("p (gh c) -> p gh c", c=C)
                )
                sa = small.tile([128, GH], dt)
                nc.vector.tensor_mul(sa, rstd[:, :, 0], rstd[:, :, 1])
                sb = small.tile([128, GH], dt)
                nc.vector.tensor_mul(sb, rstd[:, :, 2], rstd[:, :, 3])
                sab = small.tile([128, GH], dt)
                nc.vector.tensor_mul(sab, sa, sb)
                scl2 = small.tile([128, GH], dt)
                nc.vector.tensor_mul(scl2, sab, gpair[:, h0 : h0 + GH])

                # ---- group-batched residual 0.25*sum_c x (transposed space)
                rt1 = rpool.tile([128, 1024], dt, tag="ra")
                nc.vector.tensor_add(rt1, xt_g[:, 0], xt_g[:, 1])
                rt2 = rpool.tile([128, 1024], dt, tag="rb")
                nc.vector.tensor_add(rt2, xt_g[:, 2], xt_g[:, 3])
                rts = rpool.tile([128, 1024], dt, tag="rb")
                nc.vector.tensor_add(rts, rt1, rt2)
                xq = rpool.tile([128, 1024], dt, tag="ra")
                nc.vector.tensor_scalar_mul(out=xq, in0=rts, scalar1=0.25)

                # ---- group-batched merged = P0*P1*P2*P3 * scl2
                m01 = work.tile([128, GH, R], dt)
                nc.vector.tensor_mul(m01, ppall[:, :, 0, :], ppall[:, :, 1, :])
                m23 = work.tile([128, GH, R], dt)
                nc.vector.tensor_mul(m23, ppall[:, :, 2, :], ppall[:, :, 3, :])
                mgall = work.tile([128, GH, R], dt)
                nc.vector.tensor_mul(mgall, m01, m23)
                scl2b = scl2.unsqueeze(2).broadcast_to([128, GH, R])
                nc.vector.tensor_mul(mgall, mgall, scl2b)

                # ---- phase 2: transpose merged + second matmul, two
                # pairs per output PSUM bank
                for ghp in range(0, GH, 2):
                    op = ops.tile([128, 2, D], dt)
                    for dg in range(2):
                        gh = ghp + dg
                        mtp = mtps.tile([64, 128], dt)
                        nc.tensor.matmul(mtp, lhsT=mgall[:, gh, :], rhs=identity)
                        mt = work.tile([64, 128], dt)
                        nc.scalar.copy(out=mt, in_=mtp)
                        for g2 in range(2):
                            j = 2 * gh + g2
                            u8, j8 = j // 8, j % 8
                            nc.tensor.matmul(
                                op[:, dg, 64 * g2 : 64 * g2 + 64],
                                lhsT=fo_t[:, u8, 128 * j8 : 128 * j8 + 128],
                                rhs=mt[:, 64 * g2 : 64 * g2 + 64],
                            )

                    nc.vector.tensor_add(
                        o_t[:, ghp : ghp + 2, :],
                        op,
                        xq[:, 128 * ghp : 128 * ghp + 256].rearrange(
                            "p (two o) -> p two o", two=2
                        ),
                    )

                # single full-width packed store, 4KiB runs
                nc.sync.dma_start(
                    out=out[gi], in_=o_t.rearrange("p gh o -> p (gh o)")
                )

            if repeat > 1:
                with tc.For_i(0, repeat, 1):
                    emit_all_groups()
            else:
                emit_all_groups()

    nc.compile()
    return nc


def _get_nc(repeat=1):
    key = ("nc", repeat)
    if key not in _CACHE:
        _CACHE[key] = _build_nc(repeat)
    return _CACHE[key]


def _pack_x(x):
    # [B, N, C, D] -> [N//16, C, D, 1024] ; n = g*16 + j, col = j*64 + b
    a = x.reshape(B, N // 16, 16, C, D)
    a = np.transpose(a, (1, 3, 4, 2, 0))  # [g, c, i, j, b]
    return np.ascontiguousarray(a.reshape(N // 16, C, D, 1024))


def _pack_factors(factors):
    # [4, N, R, D] -> [C, N//16, D, 1024]
    f = factors.reshape(C, N // 16, 16, R, D)
    f = np.transpose(f, (0, 1, 4, 2, 3))  # [c, g, i, j, r]
    return np.ascontiguousarray(f.reshape(C, N // 16, D, 1024))


def _pack_factor_out(factor_out):
    # [N, R, D] -> [N//8, R, 8*D]
    q = factor_out.reshape(N // 8, 8, R, D)
    q = np.transpose(q, (0, 2, 1, 3))  # [oct, r, node8, o]
    return np.ascontiguousarray(q.reshape(N // 8, R, 8 * D))


def _unpack_out(res_t):
    # [NG, 128(o), GH*D] with col = gh*128 + g2*64 + b -> [B, NS, D]
    a = res_t.reshape(NG, 128, GH, 2, 64)  # [gi, o, gh, g2, b]
    a = np.transpose(a, (4, 0, 2, 3, 1))  # [b, gi, gh, g2, o]
    return np.ascontiguousarray(a.reshape(64, NS, D))


def kernel(x, factors, factor_out, gain):
    from concourse.bass_utils import run_bass_kernel_spmd

    nc = _get_nc()
    x_packed = _pack_x(np.asarray(x))
    f_packed = _pack_factors(np.asarray(factors))
    fo_packed = _pack_factor_out(np.asarray(factor_out))
    in_maps = []
    for k in range(NCORES):
        lo, hi = k * NS, (k + 1) * NS
        in_maps.append(
            {
                "xp": np.ascontiguousarray(x_packed[k * NG : (k + 1) * NG]),
                "factors_t": np.ascontiguousarray(f_packed[:, k * NG : (k + 1) * NG]),
                "factor_out_t": np.ascontiguousarray(
                    fo_packed[k * OCT : (k + 1) * OCT]
                ),
                "gain": np.ascontiguousarray(gain[lo:hi]),
            }
        )
    res = run_bass_kernel_spmd(nc, in_maps, core_ids=list(range(NCORES)))
    return np.concatenate(
        [_unpack_out(res.results[k]["out_t"]) for k in range(NCORES)], axis=1
    )
